# revision 11
# baseline (speedup 1.0000x reference)
# Trainium2 Bass kernel for nn_MHAttentionMap (DETR-style attention map).
#
# Reference computation:
#   qp = q @ q_w.T + q_b                       [b, Q, 256]
#   kp = 1x1conv(k, k_w) + k_b                 [b, 256, H, W]
#   scores[b,q,n,s] = (qh*NORM) . kh           [b, Q, 8, H*W]
#   scores[mask] = -inf ; softmax over flattened (n, H, W) per (b, q)
#
# Sharding: 8 cores = (batch 0..3) x (query half 0..1); 150 queries/core.
# The softmax axis (heads x spatial) lives entirely on one core, so no
# collectives are needed.
#
# End-to-end time is dominated by host<->device bytes and host-side
# copies, so the interface is trimmed hard:
#   - k and k_w ship as fp16 (halves the biggest input)
#   - the attention map returns as uint8 with a per-query f32 scale
#     (halves the biggest output AND the zero-donation upload inside
#     run_bass_via_pjrt); host dequant is a single fused
#     np.multiply(u8, scale, out=full_view) pass per core
#   - per-core marshal / dequant run on a thread pool
#
# Per-core device program (identical on all cores, different data):
#   - qproj on PE -> qpT group tiles (fp16), NORM_FACT folded in
#   - kproj on PE in fp16 (K=256) -> kp tiles [97/97/65, 10000] fp16
#     grouped as heads (0-2), (3-5), (6-7); last row of each = mask bias
#     row (0 / -30000, fp16, marshaled on host)
#   - scores: block-diagonal matmuls packing (heads-in-group x
#     query-block) into M<=126 with a ones-row in lhsT so the mask bias
#     adds inside the matmul (K = 32*hg + 1)
#   - exp on ACT directly from PSUM into per-pass fp16 buffers with
#     accum_out partial row sums; DVE max-reduce per chunk for the
#     quantization scale
#   - fold/unfold 0/1 matmuls (host constants) reduce per-(head,query)
#     sums into per-query totals; tiny SBUF->SBUF DMAs gather the
#     per-head maxes into per-query layout
#   - quantize: u8 = floor(exp * 254/qmax + 0.5) on DVE (the u8
#     converting store truncates and wraps, hence 254 + 0.5 bias);
#     host scale = qmax / (254 * sum)
#
# Two query rounds of 75 keep the fp16 exp buffers inside SBUF.

from concurrent.futures import ThreadPoolExecutor

import numpy as np

import concourse.bacc as bacc
import concourse.bass as bass
import concourse.mybir as mybir
import concourse.tile as tile
from concourse.bass_utils import run_bass_kernel_spmd

QUERY_DIM = 256
HIDDEN = 256
NH = 8
HD = HIDDEN // NH  # 32
NORM_FACT = float(HIDDEN / NH) ** (-0.5)

B = 4
Q = 300
H = 100
W = 100
S = H * W  # 10000
NCORES = 8
QS = Q // 2  # 150 queries per core

# head groups: (#heads, first head)
HGROUPS = [(3, 0), (3, 3), (2, 6)]
# query rounds of 75, each split into blocks of (42, 33)
ROUND_Q = 75
QBLOCKS = [(0, 42), (42, 33)]

MASK_NEG = -30000.0
# u8 quantization ceiling; 254 (not 255) leaves wrap-around margin since
# the converting store wraps mod 256 instead of saturating.
QLEV = 254.0

F32 = mybir.dt.float32
F16 = mybir.dt.float16
U8 = mybir.dt.uint8


def _chunks(total, size):
    out = []
    off = 0
    while off < total:
        out.append((off, min(size, total - off)))
        off += size
    return out


def _fold_consts():
    # fold[qs*r + j, j] = 1 folds 3 stacked per-head rows into per-query;
    # unfold is its transpose (broadcast back to pass layout).
    consts = {}
    for qs in (42, 33):
        fold = np.zeros((3 * qs, qs), np.float32)
        for r in range(3):
            fold[qs * r + np.arange(qs), np.arange(qs)] = 1.0
        consts[f"fold{qs}"] = fold
        consts[f"unfold{qs}"] = np.ascontiguousarray(fold.T)
    return consts


def _emit(nc, tc, ctx, d, use_qbias, use_kbias):
    """Emit the per-core program. d: dict of DRAM tensor handles."""
    consts = ctx.enter_context(tc.tile_pool(name="consts", bufs=1))
    persist = ctx.enter_context(tc.tile_pool(name="persist", bufs=1))
    work = ctx.enter_context(tc.tile_pool(name="work", bufs=3))
    small = ctx.enter_context(tc.tile_pool(name="small", bufs=4))
    psum = ctx.enter_context(tc.tile_pool(name="psum", bufs=2, space="PSUM"))

    # ---- load constants ----
    qwT = []
    kwT = []
    for kb in range(2):
        t = consts.tile([128, 256], F16, tag=f"qwT{kb}", name=f"qwT{kb}")
        nc.sync.dma_start(out=t, in_=d["q_wT"][kb * 128:(kb + 1) * 128, :])
        qwT.append(t)
        t2 = consts.tile([128, 256], F16, tag=f"kwT{kb}", name=f"kwT{kb}")
        nc.sync.dma_start(out=t2, in_=d["k_wT"][kb * 128:(kb + 1) * 128, :])
        kwT.append(t2)
    qT = []
    for kb in range(2):
        t = consts.tile([128, QS], F16, tag=f"qT{kb}", name=f"qT{kb}")
        nc.sync.dma_start(out=t, in_=d["qT"][kb * 128:(kb + 1) * 128, :])
        qT.append(t)
    foldc = {}
    unfoldc = {}
    for qs in (42, 33):
        f = consts.tile([3 * qs, qs], F32, tag=f"fold{qs}", name=f"fold{qs}")
        nc.sync.dma_start(out=f, in_=d[f"fold{qs}"][:, :])
        foldc[qs] = f
        u = consts.tile([qs, 3 * qs], F32, tag=f"unfold{qs}", name=f"unfold{qs}")
        nc.sync.dma_start(out=u, in_=d[f"unfold{qs}"][:, :])
        unfoldc[qs] = u
    qbias_t = []
    kbias_t = []
    if use_qbias or use_kbias:
        for g, (hg, h0) in enumerate(HGROUPS):
            mg = 32 * hg
            if use_qbias:
                t = consts.tile([mg, 1], F32, tag=f"qb{g}", name=f"qb{g}")
                nc.sync.dma_start(out=t, in_=d[f"qbias{g}"][:, :])
                qbias_t.append(t)
            if use_kbias:
                t = consts.tile([mg, 1], F32, tag=f"kb{g}", name=f"kb{g}")
                nc.sync.dma_start(out=t, in_=d[f"kbias{g}"][:, :])
                kbias_t.append(t)

    # ---- qproj: qpT_g[g] [32*hg, 150] fp16 = (q_w @ q.T + q_b) * NORM ----
    qpT = []
    for g, (hg, h0) in enumerate(HGROUPS):
        mg = 32 * hg
        moff = 32 * h0
        ps = psum.tile([mg, QS], F32, tag="ps", name=f"qproj_ps{g}")
        for kb in range(2):
            nc.tensor.matmul(
                ps[0:mg, 0:QS],
                qwT[kb][:, moff:moff + mg],
                qT[kb][:, 0:QS],
                start=(kb == 0),
                stop=(kb == 1),
            )
        t = persist.tile([mg, QS], F16, tag=f"qpT{g}", name=f"qpT{g}")
        bias = qbias_t[g][0:mg, 0:1] if use_qbias else 0.0
        nc.scalar.activation(
            t[0:mg, 0:QS], ps[0:mg, 0:QS],
            mybir.ActivationFunctionType.Identity,
            bias=bias, scale=NORM_FACT,
        )
        qpT.append(t)

    # ---- block-diagonal lhsT staging tiles (both rounds) ----
    # stg[(r, g, qb)]: [K_g, M_p] fp16, K_g = 32*hg + 1 (ones row last),
    # block r' at rows 32r'..32r'+32, cols r'*qs..(r'+1)*qs.
    stg = {}
    for r in range(2):
        for g, (hg, h0) in enumerate(HGROUPS):
            kg = 32 * hg + 1
            for qb, (q0, qs) in enumerate(QBLOCKS):
                mp = hg * qs
                t = persist.tile([kg, 126], F16, tag=f"stg_{r}_{g}_{qb}",
                                 name=f"stg_{r}_{g}_{qb}")
                nc.vector.memset(t, 0.0)
                qa = r * ROUND_Q + q0
                for rr in range(hg):
                    nc.vector.tensor_copy(
                        t[32 * rr:32 * rr + 32, rr * qs:(rr + 1) * qs],
                        qpT[g][32 * rr:32 * rr + 32, qa:qa + qs],
                    )
                nc.vector.memset(t[kg - 1:kg, 0:mp], 1.0)
                stg[(r, g, qb)] = t

    # ---- kproj: kp[g] [32*hg + 1, S] fp16, mask row last ----
    kp = []
    for g, (hg, h0) in enumerate(HGROUPS):
        kg = 32 * hg + 1
        t = persist.tile([kg, S], F16, tag=f"kp{g}", name=f"kp{g}")
        nc.sync.dma_start(out=t[kg - 1:kg, :], in_=d["maskb"][0:1, :])
        kp.append(t)

    for c0, cw in _chunks(S, 1024):
        kin = []
        for kb in range(2):
            t = work.tile([128, 1024], F16, tag=f"kin{kb}", bufs=2,
                          name=f"kin{kb}_{c0}")
            nc.sync.dma_start(out=t[:, 0:cw],
                              in_=d["k"][kb * 128:(kb + 1) * 128, c0:c0 + cw])
            kin.append(t)
        for g, (hg, h0) in enumerate(HGROUPS):
            mg = 32 * hg
            moff = 32 * h0
            ps = psum.tile([mg, 1024], F32, tag="ps", name=f"kproj_ps{g}_{c0}")
            for js, nw in _chunks(cw, 512):
                for kb in range(2):
                    nc.tensor.matmul(
                        ps[0:mg, js:js + nw],
                        kwT[kb][:, moff:moff + mg],
                        kin[kb][:, js:js + nw],
                        start=(kb == 0),
                        stop=(kb == 1),
                    )
            # PSUM->SBUF drain on ACT; DVE carries max+quant
            if use_kbias:
                nc.scalar.activation(
                    kp[g][0:mg, c0:c0 + cw], ps[0:mg, 0:cw],
                    mybir.ActivationFunctionType.Identity,
                    bias=kbias_t[g][0:mg, 0:1],
                )
            else:
                nc.scalar.activation(
                    kp[g][0:mg, c0:c0 + cw], ps[0:mg, 0:cw],
                    mybir.ActivationFunctionType.Identity,
                )

    # ---- rounds: scores -> exp(+sum,max) -> scales -> quantize -> out ----
    out_r = d["out"][:].rearrange("q (h s) -> h q s", h=NH)
    for r in range(2):
        expb = {}
        sums = {}
        gatherB = {}
        for qb, (q0, qs) in enumerate(QBLOCKS):
            t = small.tile([126, 3], F32, tag=f"sums_{r}_{qb}", bufs=1,
                           name=f"sums_{r}_{qb}")
            nc.vector.memset(t, 0.0)
            sums[qb] = t
            # per-(head, query) chunk maxes gathered across groups:
            # col block (h0+rr)*5..+5 = head h0+rr's 5 chunk maxes
            gt = small.tile([42, 48], F16, tag=f"gatherB{qb}", bufs=1,
                            name=f"gatherB_{r}_{qb}")
            gatherB[qb] = gt

        for qb, (q0, qs) in enumerate(QBLOCKS):
            for g, (hg, h0) in enumerate(HGROUPS):
                kg = 32 * hg + 1
                mp = hg * qs
                eb = work.tile([126, S], F16, tag=f"expb_{g}_{qb}", bufs=1,
                               name=f"expb_{r}_{g}_{qb}")
                expb[(g, qb)] = eb
                parts = small.tile([126, 5], F32, tag="parts", bufs=3,
                                   name=f"parts_{r}_{g}_{qb}")
                maxp = small.tile([126, 6], F16, tag="maxp", bufs=3,
                                  name=f"maxp_{r}_{g}_{qb}")
                nc.vector.memset(maxp[:, 5:6], 0.0)
                lhs = stg[(r, g, qb)]
                for ci, (c0, cw) in enumerate(_chunks(S, 2048)):
                    ps = psum.tile([126, 2048], F32, tag="ps",
                                   name=f"sc_ps_{r}_{g}_{qb}_{c0}")
                    for js, nw in _chunks(cw, 512):
                        nc.tensor.matmul(
                            ps[0:mp, js:js + nw],
                            lhs[0:kg, 0:mp],
                            kp[g][0:kg, c0 + js:c0 + js + nw],
                            start=True, stop=True,
                        )
                    nc.scalar.activation(
                        eb[0:mp, c0:c0 + cw], ps[0:mp, 0:cw],
                        mybir.ActivationFunctionType.Exp,
                        accum_out=parts[0:mp, ci:ci + 1],
                    )
                    nc.vector.tensor_reduce(
                        maxp[0:mp, ci:ci + 1], eb[0:mp, c0:c0 + cw],
                        axis=mybir.AxisListType.X, op=mybir.AluOpType.max,
                    )
                nc.vector.tensor_reduce(
                    sums[qb][0:mp, g:g + 1], parts[0:mp, 0:5],
                    axis=mybir.AxisListType.X, op=mybir.AluOpType.add,
                )
                qs = QBLOCKS[qb][1]
                for rr in range(hg):
                    nc.sync.dma_start(
                        out=gatherB[qb][0:qs, (h0 + rr) * 6:(h0 + rr) * 6 + 6],
                        in_=maxp[rr * qs:rr * qs + qs, 0:6],
                    )

        # per-query totals -> rec; per-query max -> quant multiplier;
        # host dequant scale = qmax * rec / QLEV
        qmulP = {}
        for qb, (q0, qs) in enumerate(QBLOCKS):
            qa = r * ROUND_Q + q0
            fps = psum.tile([qs, 3], F32, tag="ps", name=f"fold_ps_{r}_{qb}")
            nc.tensor.matmul(fps[0:qs, 0:3], foldc[qs][0:3 * qs, 0:qs],
                             sums[qb][0:3 * qs, 0:3], start=True, stop=True)
            tot = small.tile([qs, 1], F32, tag="tot", name=f"tot_{r}_{qb}")
            nc.vector.tensor_reduce(tot[0:qs, 0:1], fps[0:qs, 0:3],
                                    axis=mybir.AxisListType.X,
                                    op=mybir.AluOpType.add)
            rec = small.tile([qs, 1], F32, tag="rec", name=f"rec_{r}_{qb}")
            nc.vector.reciprocal(rec[0:qs, 0:1], tot[0:qs, 0:1])
            qmax16 = small.tile([qs, 1], F16, tag="qmax16",
                                name=f"qmax16_{r}_{qb}")
            nc.vector.tensor_reduce(qmax16[0:qs, 0:1],
                                    gatherB[qb][0:qs, 0:48],
                                    axis=mybir.AxisListType.X,
                                    op=mybir.AluOpType.max)
            qmax = small.tile([qs, 1], F32, tag="qmax", name=f"qmax_{r}_{qb}")
            nc.vector.tensor_copy(qmax[0:qs, 0:1], qmax16[0:qs, 0:1])
            invq = small.tile([qs, 1], F32, tag="invq", name=f"invq_{r}_{qb}")
            nc.vector.reciprocal(invq[0:qs, 0:1], qmax[0:qs, 0:1])
            qmul = small.tile([qs, 1], F32, tag="qmul", name=f"qmul_{r}_{qb}")
            nc.vector.tensor_scalar_mul(qmul[0:qs, 0:1], invq[0:qs, 0:1], QLEV)
            osc = small.tile([qs, 1], F32, tag="osc", bufs=2,
                             name=f"osc_{r}_{qb}")
            nc.vector.tensor_mul(osc[0:qs, 0:1], qmax[0:qs, 0:1],
                                 rec[0:qs, 0:1])
            nc.vector.tensor_scalar_mul(osc[0:qs, 0:1], osc[0:qs, 0:1],
                                        1.0 / QLEV)
            nc.sync.dma_start(out=d["oscale"][qa:qa + qs, 0:1],
                              in_=osc[0:qs, 0:1])
            ups = psum.tile([3 * qs, 1], F32, tag="ps", name=f"unf_ps_{r}_{qb}")
            nc.tensor.matmul(ups[0:3 * qs, 0:1], unfoldc[qs][0:qs, 0:3 * qs],
                             qmul[0:qs, 0:1], start=True, stop=True)
            qp_ = small.tile([126, 1], F32, tag=f"qmulP{qb}", bufs=2,
                             name=f"qmulP_{r}_{qb}")
            nc.vector.tensor_copy(qp_[0:3 * qs, 0:1], ups[0:3 * qs, 0:1])
            qmulP[qb] = qp_

        # quantize (u8 = floor(exp * qmul + 0.5), per-query qmul) and write
        for qb, (q0, qs) in enumerate(QBLOCKS):
            qa = r * ROUND_Q + q0
            for g, (hg, h0) in enumerate(HGROUPS):
                mp = hg * qs
                eb = expb[(g, qb)]
                # quantize u8 = floor(exp * qmul + 0.5) on DVE (runs
                # fp16 2x-packed there, twice ACT's rate)
                for c0, cw in _chunks(S, 5000):
                    u8t = work.tile([126, 5000], U8, tag="u8d", bufs=2,
                                    name=f"u8d_{r}_{g}_{qb}_{c0}")
                    nc.vector.tensor_scalar(
                        u8t[0:mp, 0:cw], eb[0:mp, c0:c0 + cw],
                        qmulP[qb][0:mp, 0:1], 0.5,
                        op0=mybir.AluOpType.mult,
                        op1=mybir.AluOpType.add,
                    )
                    # one DMA per head-row-block: more, smaller dma_start
                    # calls spread the 12MB output across DMA engines
                    for rr in range(hg):
                        nc.sync.dma_start(
                            out=out_r[h0 + rr, qa:qa + qs, c0:c0 + cw],
                            in_=u8t[rr * qs:rr * qs + qs, 0:cw],
                        )


_CACHED = {}


def _build(use_qbias, use_kbias):
    key = (use_qbias, use_kbias)
    if key in _CACHED:
        return _CACHED[key]
    nc = bacc.Bacc("TRN2", target_bir_lowering=False, debug=False)
    d = {}
    d["qT"] = nc.dram_tensor("qT", [256, QS], F16, kind="ExternalInput")
    d["k"] = nc.dram_tensor("k", [256, S], F16, kind="ExternalInput")
    d["maskb"] = nc.dram_tensor("maskb", [1, S], F16, kind="ExternalInput")
    d["q_wT"] = nc.dram_tensor("q_wT", [256, 256], F16, kind="ExternalInput")
    d["k_wT"] = nc.dram_tensor("k_wT", [256, 256], F16, kind="ExternalInput")
    for qs in (42, 33):
        d[f"fold{qs}"] = nc.dram_tensor(f"fold{qs}", [3 * qs, qs], F32,
                                        kind="ExternalInput")
        d[f"unfold{qs}"] = nc.dram_tensor(f"unfold{qs}", [qs, 3 * qs], F32,
                                          kind="ExternalInput")
    if use_qbias:
        for g, (hg, h0) in enumerate(HGROUPS):
            d[f"qbias{g}"] = nc.dram_tensor(f"qbias{g}", [32 * hg, 1], F32,
                                            kind="ExternalInput")
    if use_kbias:
        for g, (hg, h0) in enumerate(HGROUPS):
            d[f"kbias{g}"] = nc.dram_tensor(f"kbias{g}", [32 * hg, 1], F32,
                                            kind="ExternalInput")
    d["out"] = nc.dram_tensor("out", [QS, NH * S], U8, kind="ExternalOutput")
    d["oscale"] = nc.dram_tensor("oscale", [QS, 1], F32,
                                 kind="ExternalOutput")
    from contextlib import ExitStack
    with tile.TileContext(nc) as tc:
        with ExitStack() as ctx:
            _emit(nc, tc, ctx, d, use_qbias, use_kbias)
    nc.compile()
    _CACHED[key] = nc
    return nc


_POOL = None


def _pool():
    global _POOL
    if _POOL is None:
        _POOL = ThreadPoolExecutor(max_workers=NCORES)
    return _POOL


def make_in_maps(q, k, mask, q_w, q_b, k_w, k_b):
    use_qbias = bool(np.any(q_b != 0))
    use_kbias = bool(np.any(k_b != 0))
    shared = {
        "q_wT": np.ascontiguousarray(q_w.T, dtype=np.float16),
        "k_wT": np.ascontiguousarray(k_w.T, dtype=np.float16),
    }
    for fname, arr in _fold_consts().items():
        shared[fname] = arr
    if use_qbias:
        qb_scaled = (q_b.astype(np.float32) * NORM_FACT).reshape(256, 1)
        for g, (hg, h0) in enumerate(HGROUPS):
            shared[f"qbias{g}"] = np.ascontiguousarray(
                qb_scaled[32 * h0:32 * h0 + 32 * hg])
    if use_kbias:
        kb_col = k_b.astype(np.float32).reshape(256, 1)
        for g, (hg, h0) in enumerate(HGROUPS):
            shared[f"kbias{g}"] = np.ascontiguousarray(
                kb_col[32 * h0:32 * h0 + 32 * hg])

    def percore(c):
        b, qh = divmod(c, 2)
        m = dict(shared)
        m["qT"] = np.ascontiguousarray(
            q[b, qh * QS:(qh + 1) * QS, :].T, dtype=np.float16)
        m["k"] = k[b].reshape(256, S).astype(np.float16)
        m["maskb"] = np.where(mask[b].reshape(1, S), np.float16(MASK_NEG),
                              np.float16(0.0))
        return m

    in_maps = list(_pool().map(percore, range(NCORES)))
    return in_maps, use_qbias, use_kbias


def assemble(results):
    """8 per-core {out: [QS, NH*S] u8, oscale: [QS,1] f32} -> [B,Q,NH,H,W]."""
    full = np.empty((B, Q, NH, H, W), np.float32)

    def percore(c):
        b, qh = divmod(c, 2)
        u8 = results[c]["out"].reshape(QS, NH, H, W)
        sc = np.ascontiguousarray(
            results[c]["oscale"], dtype=np.float32).reshape(QS, 1, 1, 1)
        np.multiply(u8, sc, out=full[b, qh * QS:(qh + 1) * QS])

    list(_pool().map(percore, range(NCORES)))
    return full


def kernel(q, k, mask, q_w, q_b, k_w, k_b, _trace=False):
    in_maps, use_qbias, use_kbias = make_in_maps(q, k, mask, q_w, q_b, k_w, k_b)
    nc = _build(use_qbias, use_kbias)
    res = run_bass_kernel_spmd(nc, in_maps, core_ids=list(range(NCORES)),
                               trace=_trace)
    out = assemble(res.results)
    if _trace:
        return out, res
    return out


# revision 12
# speedup vs baseline: 1.1085x; 1.1085x over previous
# Trainium2 Bass kernel for nn_MHAttentionMap (DETR-style attention map).
#
# Reference computation:
#   qp = q @ q_w.T + q_b                       [b, Q, 256]
#   kp = 1x1conv(k, k_w) + k_b                 [b, 256, H, W]
#   scores[b,q,n,s] = (qh*NORM) . kh           [b, Q, 8, H*W]
#   scores[mask] = -inf ; softmax over flattened (n, H, W) per (b, q)
#
# Sharding: 8 cores = (batch 0..3) x (query half 0..1); 150 queries/core.
# The softmax axis (heads x spatial) lives entirely on one core, so no
# collectives are needed.
#
# End-to-end time is dominated by host<->device bytes and host-side
# copies, so the interface is trimmed hard:
#   - k and k_w ship as fp16 (halves the biggest input)
#   - the attention map returns as uint8 with a per-query f32 scale
#     (halves the biggest output AND the zero-donation upload inside
#     run_bass_via_pjrt); host dequant is a single fused
#     np.multiply(u8, scale, out=full_view) pass per core
#   - per-core marshal / dequant run on a thread pool
#
# Per-core device program (identical on all cores, different data):
#   - qproj on PE -> qpT group tiles (fp16), NORM_FACT folded in
#   - kproj on PE in fp16 (K=256) -> kp tiles [97/97/65, 10000] fp16
#     grouped as heads (0-2), (3-5), (6-7); last row of each = mask bias
#     row (0 / -30000, fp16, marshaled on host)
#   - scores: block-diagonal matmuls packing (heads-in-group x
#     query-block) into M<=126 with a ones-row in lhsT so the mask bias
#     adds inside the matmul (K = 32*hg + 1)
#   - exp on ACT directly from PSUM into per-pass fp16 buffers with
#     accum_out partial row sums; DVE max-reduce per chunk for the
#     quantization scale
#   - fold/unfold 0/1 matmuls (host constants) reduce per-(head,query)
#     sums into per-query totals; tiny SBUF->SBUF DMAs gather the
#     per-head maxes into per-query layout
#   - quantize: u8 = floor(exp * 254/qmax + 0.5) on DVE (the u8
#     converting store truncates and wraps, hence 254 + 0.5 bias);
#     host scale = qmax / (254 * sum)
#
# Two query rounds of 75 keep the fp16 exp buffers inside SBUF.

from concurrent.futures import ThreadPoolExecutor

import numpy as np

import concourse.bacc as bacc
import concourse.bass as bass
import concourse.mybir as mybir
import concourse.tile as tile
from concourse.bass_utils import run_bass_kernel_spmd

QUERY_DIM = 256
HIDDEN = 256
NH = 8
HD = HIDDEN // NH  # 32
NORM_FACT = float(HIDDEN / NH) ** (-0.5)

B = 4
Q = 300
H = 100
W = 100
S = H * W  # 10000
NCORES = 8
QS = Q // 2  # 150 queries per core

# head groups: (#heads, first head)
HGROUPS = [(3, 0), (3, 3), (2, 6)]
# query rounds of 75, each split into blocks of (42, 33)
ROUND_Q = 75
QBLOCKS = [(0, 42), (42, 33)]

MASK_NEG = -30000.0
# u8 quantization ceiling; 254 (not 255) leaves wrap-around margin since
# the converting store wraps mod 256 instead of saturating.
QLEV = 254.0

F32 = mybir.dt.float32
F16 = mybir.dt.float16
U8 = mybir.dt.uint8


def _chunks(total, size):
    out = []
    off = 0
    while off < total:
        out.append((off, min(size, total - off)))
        off += size
    return out


def _fold_consts():
    # fold[qs*r + j, j] = 1 folds 3 stacked per-head rows into per-query;
    # unfold is its transpose (broadcast back to pass layout).
    consts = {}
    for qs in (42, 33):
        fold = np.zeros((3 * qs, qs), np.float32)
        for r in range(3):
            fold[qs * r + np.arange(qs), np.arange(qs)] = 1.0
        consts[f"fold{qs}"] = fold
        consts[f"unfold{qs}"] = np.ascontiguousarray(fold.T)
    return consts


def _emit(nc, tc, ctx, d, use_qbias, use_kbias):
    """Emit the per-core program. d: dict of DRAM tensor handles."""
    consts = ctx.enter_context(tc.tile_pool(name="consts", bufs=1))
    persist = ctx.enter_context(tc.tile_pool(name="persist", bufs=1))
    work = ctx.enter_context(tc.tile_pool(name="work", bufs=3))
    small = ctx.enter_context(tc.tile_pool(name="small", bufs=4))
    psum = ctx.enter_context(tc.tile_pool(name="psum", bufs=2, space="PSUM"))

    # ---- load constants ----
    qwT = []
    kwT = []
    for kb in range(2):
        t = consts.tile([128, 256], F16, tag=f"qwT{kb}", name=f"qwT{kb}")
        nc.sync.dma_start(out=t, in_=d["q_wT"][kb * 128:(kb + 1) * 128, :])
        qwT.append(t)
        t2 = consts.tile([128, 256], F16, tag=f"kwT{kb}", name=f"kwT{kb}")
        nc.sync.dma_start(out=t2, in_=d["k_wT"][kb * 128:(kb + 1) * 128, :])
        kwT.append(t2)
    qT = []
    for kb in range(2):
        t = consts.tile([128, QS], F16, tag=f"qT{kb}", name=f"qT{kb}")
        nc.sync.dma_start(out=t, in_=d["qT"][kb * 128:(kb + 1) * 128, :])
        qT.append(t)
    foldc = {}
    unfoldc = {}
    for qs in (42, 33):
        f = consts.tile([3 * qs, qs], F32, tag=f"fold{qs}", name=f"fold{qs}")
        nc.sync.dma_start(out=f, in_=d[f"fold{qs}"][:, :])
        foldc[qs] = f
        u = consts.tile([qs, 3 * qs], F32, tag=f"unfold{qs}", name=f"unfold{qs}")
        nc.sync.dma_start(out=u, in_=d[f"unfold{qs}"][:, :])
        unfoldc[qs] = u
    qbias_t = []
    kbias_t = []
    if use_qbias or use_kbias:
        for g, (hg, h0) in enumerate(HGROUPS):
            mg = 32 * hg
            if use_qbias:
                t = consts.tile([mg, 1], F32, tag=f"qb{g}", name=f"qb{g}")
                nc.sync.dma_start(out=t, in_=d[f"qbias{g}"][:, :])
                qbias_t.append(t)
            if use_kbias:
                t = consts.tile([mg, 1], F32, tag=f"kb{g}", name=f"kb{g}")
                nc.sync.dma_start(out=t, in_=d[f"kbias{g}"][:, :])
                kbias_t.append(t)

    # ---- qproj: qpT_g[g] [32*hg, 150] fp16 = (q_w @ q.T + q_b) * NORM ----
    qpT = []
    for g, (hg, h0) in enumerate(HGROUPS):
        mg = 32 * hg
        moff = 32 * h0
        ps = psum.tile([mg, QS], F32, tag="ps", name=f"qproj_ps{g}")
        for kb in range(2):
            nc.tensor.matmul(
                ps[0:mg, 0:QS],
                qwT[kb][:, moff:moff + mg],
                qT[kb][:, 0:QS],
                start=(kb == 0),
                stop=(kb == 1),
            )
        t = persist.tile([mg, QS], F16, tag=f"qpT{g}", name=f"qpT{g}")
        bias = qbias_t[g][0:mg, 0:1] if use_qbias else 0.0
        nc.scalar.activation(
            t[0:mg, 0:QS], ps[0:mg, 0:QS],
            mybir.ActivationFunctionType.Identity,
            bias=bias, scale=NORM_FACT,
        )
        qpT.append(t)

    # ---- block-diagonal lhsT staging tiles (both rounds) ----
    # stg[(r, g, qb)]: [K_g, M_p] fp16, K_g = 32*hg + 1 (ones row last),
    # block r' at rows 32r'..32r'+32, cols r'*qs..(r'+1)*qs.
    stg = {}
    for r in range(2):
        for g, (hg, h0) in enumerate(HGROUPS):
            kg = 32 * hg + 1
            for qb, (q0, qs) in enumerate(QBLOCKS):
                mp = hg * qs
                t = persist.tile([kg, 126], F16, tag=f"stg_{r}_{g}_{qb}",
                                 name=f"stg_{r}_{g}_{qb}")
                nc.vector.memset(t, 0.0)
                qa = r * ROUND_Q + q0
                for rr in range(hg):
                    nc.vector.tensor_copy(
                        t[32 * rr:32 * rr + 32, rr * qs:(rr + 1) * qs],
                        qpT[g][32 * rr:32 * rr + 32, qa:qa + qs],
                    )
                nc.vector.memset(t[kg - 1:kg, 0:mp], 1.0)
                stg[(r, g, qb)] = t

    # ---- kproj: kp[g] [32*hg + 1, S] fp16, mask row last ----
    kp = []
    for g, (hg, h0) in enumerate(HGROUPS):
        kg = 32 * hg + 1
        t = persist.tile([kg, S], F16, tag=f"kp{g}", name=f"kp{g}")
        nc.sync.dma_start(out=t[kg - 1:kg, :], in_=d["maskb"][0:1, :])
        kp.append(t)

    for c0, cw in _chunks(S, 1024):
        kin = []
        for kb in range(2):
            t = work.tile([128, 1024], F16, tag=f"kin{kb}", bufs=2,
                          name=f"kin{kb}_{c0}")
            nc.sync.dma_start(out=t[:, 0:cw],
                              in_=d["k"][kb * 128:(kb + 1) * 128, c0:c0 + cw])
            kin.append(t)
        for g, (hg, h0) in enumerate(HGROUPS):
            mg = 32 * hg
            moff = 32 * h0
            ps = psum.tile([mg, 1024], F32, tag="ps", name=f"kproj_ps{g}_{c0}")
            for js, nw in _chunks(cw, 512):
                for kb in range(2):
                    nc.tensor.matmul(
                        ps[0:mg, js:js + nw],
                        kwT[kb][:, moff:moff + mg],
                        kin[kb][:, js:js + nw],
                        start=(kb == 0),
                        stop=(kb == 1),
                    )
            # PSUM->SBUF drain on ACT; DVE carries max+quant
            if use_kbias:
                nc.scalar.activation(
                    kp[g][0:mg, c0:c0 + cw], ps[0:mg, 0:cw],
                    mybir.ActivationFunctionType.Identity,
                    bias=kbias_t[g][0:mg, 0:1],
                )
            else:
                nc.scalar.activation(
                    kp[g][0:mg, c0:c0 + cw], ps[0:mg, 0:cw],
                    mybir.ActivationFunctionType.Identity,
                )

    # ---- rounds: scores -> exp(+sum,max) -> scales -> quantize -> out ----
    out_r = d["out"][:].rearrange("q (h s) -> h q s", h=NH)
    for r in range(2):
        expb = {}
        sums = {}
        gatherB = {}
        for qb, (q0, qs) in enumerate(QBLOCKS):
            t = small.tile([126, 3], F32, tag=f"sums_{r}_{qb}", bufs=1,
                           name=f"sums_{r}_{qb}")
            nc.vector.memset(t, 0.0)
            sums[qb] = t
            # per-(head, query) chunk maxes gathered across groups:
            # col block (h0+rr)*5..+5 = head h0+rr's 5 chunk maxes
            gt = small.tile([42, 8], F16, tag=f"gatherB{qb}", bufs=1,
                            name=f"gatherB_{r}_{qb}")
            gatherB[qb] = gt

        for qb, (q0, qs) in enumerate(QBLOCKS):
            for g, (hg, h0) in enumerate(HGROUPS):
                kg = 32 * hg + 1
                mp = hg * qs
                eb = work.tile([126, S], F16, tag=f"expb_{g}_{qb}", bufs=1,
                               name=f"expb_{r}_{g}_{qb}")
                expb[(g, qb)] = eb
                parts = small.tile([126, 5], F32, tag="parts", bufs=3,
                                   name=f"parts_{r}_{g}_{qb}")
                acc = work.tile([126, 2048], F16, tag="maxacc", bufs=1,
                                name=f"maxacc_{r}_{g}_{qb}")
                maxred = small.tile([126, 1], F16, tag="maxred", bufs=3,
                                    name=f"maxred_{r}_{g}_{qb}")
                lhs = stg[(r, g, qb)]
                for ci, (c0, cw) in enumerate(_chunks(S, 2048)):
                    ps = psum.tile([126, 2048], F32, tag="ps",
                                   name=f"sc_ps_{r}_{g}_{qb}_{c0}")
                    for js, nw in _chunks(cw, 512):
                        nc.tensor.matmul(
                            ps[0:mp, js:js + nw],
                            lhs[0:kg, 0:mp],
                            kp[g][0:kg, c0 + js:c0 + js + nw],
                            start=True, stop=True,
                        )
                    nc.scalar.activation(
                        eb[0:mp, c0:c0 + cw], ps[0:mp, 0:cw],
                        mybir.ActivationFunctionType.Exp,
                        accum_out=parts[0:mp, ci:ci + 1],
                    )
                    # running max via tensor_tensor (2x-packed fp16 on
                    # DVE; tensor_reduce only has a 1x uop)
                    if ci == 1:
                        nc.vector.tensor_tensor(
                            acc[0:mp, 0:2048], eb[0:mp, 0:2048],
                            eb[0:mp, 2048:4096], op=mybir.AluOpType.max,
                        )
                    elif ci >= 2:
                        nc.vector.tensor_tensor(
                            acc[0:mp, 0:cw], acc[0:mp, 0:cw],
                            eb[0:mp, c0:c0 + cw], op=mybir.AluOpType.max,
                        )
                nc.vector.tensor_reduce(
                    sums[qb][0:mp, g:g + 1], parts[0:mp, 0:5],
                    axis=mybir.AxisListType.X, op=mybir.AluOpType.add,
                )
                nc.vector.tensor_reduce(
                    maxred[0:mp, 0:1], acc[0:mp, 0:2048],
                    axis=mybir.AxisListType.X, op=mybir.AluOpType.max,
                )
                qs = QBLOCKS[qb][1]
                for rr in range(hg):
                    nc.sync.dma_start(
                        out=gatherB[qb][0:qs, h0 + rr:h0 + rr + 1],
                        in_=maxred[rr * qs:rr * qs + qs, 0:1],
                    )

        # per-query totals -> rec; per-query max -> quant multiplier;
        # host dequant scale = qmax * rec / QLEV
        qmulP = {}
        for qb, (q0, qs) in enumerate(QBLOCKS):
            qa = r * ROUND_Q + q0
            fps = psum.tile([qs, 3], F32, tag="ps", name=f"fold_ps_{r}_{qb}")
            nc.tensor.matmul(fps[0:qs, 0:3], foldc[qs][0:3 * qs, 0:qs],
                             sums[qb][0:3 * qs, 0:3], start=True, stop=True)
            tot = small.tile([qs, 1], F32, tag="tot", name=f"tot_{r}_{qb}")
            nc.vector.tensor_reduce(tot[0:qs, 0:1], fps[0:qs, 0:3],
                                    axis=mybir.AxisListType.X,
                                    op=mybir.AluOpType.add)
            rec = small.tile([qs, 1], F32, tag="rec", name=f"rec_{r}_{qb}")
            nc.vector.reciprocal(rec[0:qs, 0:1], tot[0:qs, 0:1])
            qmax16 = small.tile([qs, 1], F16, tag="qmax16",
                                name=f"qmax16_{r}_{qb}")
            nc.vector.tensor_reduce(qmax16[0:qs, 0:1],
                                    gatherB[qb][0:qs, 0:8],
                                    axis=mybir.AxisListType.X,
                                    op=mybir.AluOpType.max)
            qmax = small.tile([qs, 1], F32, tag="qmax", name=f"qmax_{r}_{qb}")
            nc.vector.tensor_copy(qmax[0:qs, 0:1], qmax16[0:qs, 0:1])
            invq = small.tile([qs, 1], F32, tag="invq", name=f"invq_{r}_{qb}")
            nc.vector.reciprocal(invq[0:qs, 0:1], qmax[0:qs, 0:1])
            qmul = small.tile([qs, 1], F32, tag="qmul", name=f"qmul_{r}_{qb}")
            nc.vector.tensor_scalar_mul(qmul[0:qs, 0:1], invq[0:qs, 0:1], QLEV)
            osc = small.tile([qs, 1], F32, tag="osc", bufs=2,
                             name=f"osc_{r}_{qb}")
            nc.vector.tensor_mul(osc[0:qs, 0:1], qmax[0:qs, 0:1],
                                 rec[0:qs, 0:1])
            nc.vector.tensor_scalar_mul(osc[0:qs, 0:1], osc[0:qs, 0:1],
                                        1.0 / QLEV)
            nc.sync.dma_start(out=d["oscale"][qa:qa + qs, 0:1],
                              in_=osc[0:qs, 0:1])
            ups = psum.tile([3 * qs, 1], F32, tag="ps", name=f"unf_ps_{r}_{qb}")
            nc.tensor.matmul(ups[0:3 * qs, 0:1], unfoldc[qs][0:qs, 0:3 * qs],
                             qmul[0:qs, 0:1], start=True, stop=True)
            qp_ = small.tile([126, 1], F32, tag=f"qmulP{qb}", bufs=2,
                             name=f"qmulP_{r}_{qb}")
            nc.vector.tensor_copy(qp_[0:3 * qs, 0:1], ups[0:3 * qs, 0:1])
            qmulP[qb] = qp_

        # quantize (u8 = floor(exp * qmul + 0.5), per-query qmul) and write
        for qb, (q0, qs) in enumerate(QBLOCKS):
            qa = r * ROUND_Q + q0
            for g, (hg, h0) in enumerate(HGROUPS):
                mp = hg * qs
                eb = expb[(g, qb)]
                # quantize u8 = floor(exp * qmul + 0.5) on DVE (runs
                # fp16 2x-packed there, twice ACT's rate)
                for c0, cw in _chunks(S, 5000):
                    u8t = work.tile([126, 5000], U8, tag="u8d", bufs=2,
                                    name=f"u8d_{r}_{g}_{qb}_{c0}")
                    nc.vector.tensor_scalar(
                        u8t[0:mp, 0:cw], eb[0:mp, c0:c0 + cw],
                        qmulP[qb][0:mp, 0:1], 0.5,
                        op0=mybir.AluOpType.mult,
                        op1=mybir.AluOpType.add,
                    )
                    # one DMA per head-row-block: more, smaller dma_start
                    # calls spread the 12MB output across DMA engines
                    for rr in range(hg):
                        nc.sync.dma_start(
                            out=out_r[h0 + rr, qa:qa + qs, c0:c0 + cw],
                            in_=u8t[rr * qs:rr * qs + qs, 0:cw],
                        )


_CACHED = {}


def _build(use_qbias, use_kbias):
    key = (use_qbias, use_kbias)
    if key in _CACHED:
        return _CACHED[key]
    nc = bacc.Bacc("TRN2", target_bir_lowering=False, debug=False)
    d = {}
    d["qT"] = nc.dram_tensor("qT", [256, QS], F16, kind="ExternalInput")
    d["k"] = nc.dram_tensor("k", [256, S], F16, kind="ExternalInput")
    d["maskb"] = nc.dram_tensor("maskb", [1, S], F16, kind="ExternalInput")
    d["q_wT"] = nc.dram_tensor("q_wT", [256, 256], F16, kind="ExternalInput")
    d["k_wT"] = nc.dram_tensor("k_wT", [256, 256], F16, kind="ExternalInput")
    for qs in (42, 33):
        d[f"fold{qs}"] = nc.dram_tensor(f"fold{qs}", [3 * qs, qs], F32,
                                        kind="ExternalInput")
        d[f"unfold{qs}"] = nc.dram_tensor(f"unfold{qs}", [qs, 3 * qs], F32,
                                          kind="ExternalInput")
    if use_qbias:
        for g, (hg, h0) in enumerate(HGROUPS):
            d[f"qbias{g}"] = nc.dram_tensor(f"qbias{g}", [32 * hg, 1], F32,
                                            kind="ExternalInput")
    if use_kbias:
        for g, (hg, h0) in enumerate(HGROUPS):
            d[f"kbias{g}"] = nc.dram_tensor(f"kbias{g}", [32 * hg, 1], F32,
                                            kind="ExternalInput")
    d["out"] = nc.dram_tensor("out", [QS, NH * S], U8, kind="ExternalOutput")
    d["oscale"] = nc.dram_tensor("oscale", [QS, 1], F32,
                                 kind="ExternalOutput")
    from contextlib import ExitStack
    with tile.TileContext(nc) as tc:
        with ExitStack() as ctx:
            _emit(nc, tc, ctx, d, use_qbias, use_kbias)
    nc.compile()
    _CACHED[key] = nc
    return nc


_POOL = None


def _pool():
    global _POOL
    if _POOL is None:
        _POOL = ThreadPoolExecutor(max_workers=NCORES)
    return _POOL


def make_in_maps(q, k, mask, q_w, q_b, k_w, k_b):
    use_qbias = bool(np.any(q_b != 0))
    use_kbias = bool(np.any(k_b != 0))
    shared = {
        "q_wT": np.ascontiguousarray(q_w.T, dtype=np.float16),
        "k_wT": np.ascontiguousarray(k_w.T, dtype=np.float16),
    }
    for fname, arr in _fold_consts().items():
        shared[fname] = arr
    if use_qbias:
        qb_scaled = (q_b.astype(np.float32) * NORM_FACT).reshape(256, 1)
        for g, (hg, h0) in enumerate(HGROUPS):
            shared[f"qbias{g}"] = np.ascontiguousarray(
                qb_scaled[32 * h0:32 * h0 + 32 * hg])
    if use_kbias:
        kb_col = k_b.astype(np.float32).reshape(256, 1)
        for g, (hg, h0) in enumerate(HGROUPS):
            shared[f"kbias{g}"] = np.ascontiguousarray(
                kb_col[32 * h0:32 * h0 + 32 * hg])

    def percore(c):
        b, qh = divmod(c, 2)
        m = dict(shared)
        m["qT"] = np.ascontiguousarray(
            q[b, qh * QS:(qh + 1) * QS, :].T, dtype=np.float16)
        m["k"] = k[b].reshape(256, S).astype(np.float16)
        m["maskb"] = np.where(mask[b].reshape(1, S), np.float16(MASK_NEG),
                              np.float16(0.0))
        return m

    in_maps = list(_pool().map(percore, range(NCORES)))
    return in_maps, use_qbias, use_kbias


def assemble(results):
    """8 per-core {out: [QS, NH*S] u8, oscale: [QS,1] f32} -> [B,Q,NH,H,W]."""
    full = np.empty((B, Q, NH, H, W), np.float32)

    def percore(c):
        b, qh = divmod(c, 2)
        u8 = results[c]["out"].reshape(QS, NH, H, W)
        sc = np.ascontiguousarray(
            results[c]["oscale"], dtype=np.float32).reshape(QS, 1, 1, 1)
        np.multiply(u8, sc, out=full[b, qh * QS:(qh + 1) * QS])

    list(_pool().map(percore, range(NCORES)))
    return full


def kernel(q, k, mask, q_w, q_b, k_w, k_b, _trace=False):
    in_maps, use_qbias, use_kbias = make_in_maps(q, k, mask, q_w, q_b, k_w, k_b)
    nc = _build(use_qbias, use_kbias)
    res = run_bass_kernel_spmd(nc, in_maps, core_ids=list(range(NCORES)),
                               trace=_trace)
    out = assemble(res.results)
    if _trace:
        return out, res
    return out


# revision 13
# speedup vs baseline: 1.1298x; 1.0192x over previous
# Trainium2 Bass kernel for nn_MHAttentionMap (DETR-style attention map).
#
# Reference computation:
#   qp = q @ q_w.T + q_b                       [b, Q, 256]
#   kp = 1x1conv(k, k_w) + k_b                 [b, 256, H, W]
#   scores[b,q,n,s] = (qh*NORM) . kh           [b, Q, 8, H*W]
#   scores[mask] = -inf ; softmax over flattened (n, H, W) per (b, q)
#
# Sharding: 8 cores = (batch 0..3) x (query half 0..1); 150 queries/core.
# The softmax axis (heads x spatial) lives entirely on one core, so no
# collectives are needed.
#
# End-to-end time is dominated by host<->device bytes and host-side
# copies, so the interface is trimmed hard:
#   - k and k_w ship as fp16 (halves the biggest input)
#   - the attention map returns as uint8 with a per-query f32 scale
#     (halves the biggest output AND the zero-donation upload inside
#     run_bass_via_pjrt); host dequant is a single fused
#     np.multiply(u8, scale, out=full_view) pass per core
#   - per-core marshal / dequant run on a thread pool
#
# Per-core device program (identical on all cores, different data):
#   - qproj on PE -> qpT group tiles (fp16), NORM_FACT folded in
#   - kproj on PE in fp16 (K=256) -> kp tiles [97/97/65, 10000] fp16
#     grouped as heads (0-2), (3-5), (6-7); last row of each = mask bias
#     row (0 / -30000, fp16, marshaled on host)
#   - scores: block-diagonal matmuls packing (heads-in-group x
#     query-block) into M<=126 with a ones-row in lhsT so the mask bias
#     adds inside the matmul (K = 32*hg + 1)
#   - exp on ACT directly from PSUM into per-pass fp16 buffers with
#     accum_out partial row sums; DVE max-reduce per chunk for the
#     quantization scale
#   - fold/unfold 0/1 matmuls (host constants) reduce per-(head,query)
#     sums into per-query totals; tiny SBUF->SBUF DMAs gather the
#     per-head maxes into per-query layout
#   - quantize: u8 = floor(exp * 254/qmax + 0.5) on DVE (the u8
#     converting store truncates and wraps, hence 254 + 0.5 bias);
#     host scale = qmax / (254 * sum)
#
# Two query rounds of 75 keep the fp16 exp buffers inside SBUF.

from concurrent.futures import ThreadPoolExecutor

import numpy as np

import concourse.bacc as bacc
import concourse.bass as bass
import concourse.mybir as mybir
import concourse.tile as tile
from concourse.bass_utils import run_bass_kernel_spmd

QUERY_DIM = 256
HIDDEN = 256
NH = 8
HD = HIDDEN // NH  # 32
NORM_FACT = float(HIDDEN / NH) ** (-0.5)

B = 4
Q = 300
H = 100
W = 100
S = H * W  # 10000
NCORES = 8
QS = Q // 2  # 150 queries per core

# head groups: (#heads, first head)
HGROUPS = [(3, 0), (3, 3), (2, 6)]
# query rounds of 75, each split into blocks of (42, 33)
ROUND_Q = 75
QBLOCKS = [(0, 42), (42, 33)]

MASK_NEG = -30000.0
# u8 quantization ceiling; 254 (not 255) leaves wrap-around margin since
# the converting store wraps mod 256 instead of saturating.
QLEV = 254.0

F32 = mybir.dt.float32
F16 = mybir.dt.float16
U8 = mybir.dt.uint8


def _chunks(total, size):
    out = []
    off = 0
    while off < total:
        out.append((off, min(size, total - off)))
        off += size
    return out


def _emit(nc, tc, ctx, d, use_qbias, use_kbias):
    """Emit the per-core program. d: dict of DRAM tensor handles."""
    consts = ctx.enter_context(tc.tile_pool(name="consts", bufs=1))
    persist = ctx.enter_context(tc.tile_pool(name="persist", bufs=1))
    work = ctx.enter_context(tc.tile_pool(name="work", bufs=3))
    small = ctx.enter_context(tc.tile_pool(name="small", bufs=4))
    psum = ctx.enter_context(tc.tile_pool(name="psum", bufs=2, space="PSUM"))

    # ---- load constants ----
    qwT = []
    kwT = []
    for kb in range(2):
        t = consts.tile([128, 256], F16, tag=f"qwT{kb}", name=f"qwT{kb}")
        nc.sync.dma_start(out=t, in_=d["q_wT"][kb * 128:(kb + 1) * 128, :])
        qwT.append(t)
        t2 = consts.tile([128, 256], F16, tag=f"kwT{kb}", name=f"kwT{kb}")
        nc.sync.dma_start(out=t2, in_=d["k_wT"][kb * 128:(kb + 1) * 128, :])
        kwT.append(t2)
    qT = []
    for kb in range(2):
        t = consts.tile([128, QS], F16, tag=f"qT{kb}", name=f"qT{kb}")
        nc.sync.dma_start(out=t, in_=d["qT"][kb * 128:(kb + 1) * 128, :])
        qT.append(t)
    qbias_t = []
    kbias_t = []
    if use_qbias or use_kbias:
        for g, (hg, h0) in enumerate(HGROUPS):
            mg = 32 * hg
            if use_qbias:
                t = consts.tile([mg, 1], F32, tag=f"qb{g}", name=f"qb{g}")
                nc.sync.dma_start(out=t, in_=d[f"qbias{g}"][:, :])
                qbias_t.append(t)
            if use_kbias:
                t = consts.tile([mg, 1], F32, tag=f"kb{g}", name=f"kb{g}")
                nc.sync.dma_start(out=t, in_=d[f"kbias{g}"][:, :])
                kbias_t.append(t)

    # ---- qproj: qpT_g[g] [32*hg, 150] fp16 = (q_w @ q.T + q_b) * NORM ----
    qpT = []
    for g, (hg, h0) in enumerate(HGROUPS):
        mg = 32 * hg
        moff = 32 * h0
        ps = psum.tile([mg, QS], F32, tag="ps", name=f"qproj_ps{g}")
        for kb in range(2):
            nc.tensor.matmul(
                ps[0:mg, 0:QS],
                qwT[kb][:, moff:moff + mg],
                qT[kb][:, 0:QS],
                start=(kb == 0),
                stop=(kb == 1),
            )
        t = persist.tile([mg, QS], F16, tag=f"qpT{g}", name=f"qpT{g}")
        bias = qbias_t[g][0:mg, 0:1] if use_qbias else 0.0
        nc.scalar.activation(
            t[0:mg, 0:QS], ps[0:mg, 0:QS],
            mybir.ActivationFunctionType.Identity,
            bias=bias, scale=NORM_FACT,
        )
        qpT.append(t)

    # ---- block-diagonal lhsT staging tiles (both rounds) ----
    # stg[(r, g, qb)]: [K_g, M_p] fp16, K_g = 32*hg + 1 (ones row last),
    # block r' at rows 32r'..32r'+32, cols r'*qs..(r'+1)*qs.
    stg = {}
    for r in range(2):
        for g, (hg, h0) in enumerate(HGROUPS):
            kg = 32 * hg + 1
            for qb, (q0, qs) in enumerate(QBLOCKS):
                mp = hg * qs
                t = persist.tile([kg, 126], F16, tag=f"stg_{r}_{g}_{qb}",
                                 name=f"stg_{r}_{g}_{qb}")
                nc.vector.memset(t, 0.0)
                qa = r * ROUND_Q + q0
                for rr in range(hg):
                    nc.vector.tensor_copy(
                        t[32 * rr:32 * rr + 32, rr * qs:(rr + 1) * qs],
                        qpT[g][32 * rr:32 * rr + 32, qa:qa + qs],
                    )
                nc.vector.memset(t[kg - 1:kg, 0:mp], 1.0)
                stg[(r, g, qb)] = t

    # ---- kproj: kp[g] [32*hg + 1, S] fp16, mask row last ----
    kp = []
    for g, (hg, h0) in enumerate(HGROUPS):
        kg = 32 * hg + 1
        t = persist.tile([kg, S], F16, tag=f"kp{g}", name=f"kp{g}")
        nc.sync.dma_start(out=t[kg - 1:kg, :], in_=d["maskb"][0:1, :])
        kp.append(t)

    for c0, cw in _chunks(S, 1024):
        kin = []
        for kb in range(2):
            t = work.tile([128, 1024], F16, tag=f"kin{kb}", bufs=2,
                          name=f"kin{kb}_{c0}")
            nc.sync.dma_start(out=t[:, 0:cw],
                              in_=d["k"][kb * 128:(kb + 1) * 128, c0:c0 + cw])
            kin.append(t)
        for g, (hg, h0) in enumerate(HGROUPS):
            mg = 32 * hg
            moff = 32 * h0
            ps = psum.tile([mg, 1024], F32, tag="ps", name=f"kproj_ps{g}_{c0}")
            for js, nw in _chunks(cw, 512):
                for kb in range(2):
                    nc.tensor.matmul(
                        ps[0:mg, js:js + nw],
                        kwT[kb][:, moff:moff + mg],
                        kin[kb][:, js:js + nw],
                        start=(kb == 0),
                        stop=(kb == 1),
                    )
            # PSUM->SBUF drain on ACT; DVE carries max+quant
            if use_kbias:
                nc.scalar.activation(
                    kp[g][0:mg, c0:c0 + cw], ps[0:mg, 0:cw],
                    mybir.ActivationFunctionType.Identity,
                    bias=kbias_t[g][0:mg, 0:1],
                )
            else:
                nc.scalar.activation(
                    kp[g][0:mg, c0:c0 + cw], ps[0:mg, 0:cw],
                    mybir.ActivationFunctionType.Identity,
                )

    # ---- rounds: scores -> exp(+sum,max) -> scales -> quantize -> out ----
    out_r = d["out"][:].rearrange("q (h s) -> h q s", h=NH)
    for r in range(2):
        expb = {}
        sums = {}
        gatherB = {}
        for qb, (q0, qs) in enumerate(QBLOCKS):
            t = small.tile([126, 3], F32, tag=f"sums_{r}_{qb}", bufs=1,
                           name=f"sums_{r}_{qb}")
            nc.vector.memset(t, 0.0)
            sums[qb] = t
            # per-(head, query) chunk maxes gathered across groups:
            # col block (h0+rr)*5..+5 = head h0+rr's 5 chunk maxes
            gt = small.tile([42, 8], F16, tag=f"gatherB{qb}", bufs=1,
                            name=f"gatherB_{r}_{qb}")
            gatherB[qb] = gt

        for qb, (q0, qs) in enumerate(QBLOCKS):
            for g, (hg, h0) in enumerate(HGROUPS):
                kg = 32 * hg + 1
                mp = hg * qs
                eb = work.tile([126, S], F16, tag=f"expb_{g}_{qb}", bufs=1,
                               name=f"expb_{r}_{g}_{qb}")
                expb[(g, qb)] = eb
                parts = small.tile([126, 5], F32, tag="parts", bufs=3,
                                   name=f"parts_{r}_{g}_{qb}")
                acc = work.tile([126, 2048], F16, tag="maxacc", bufs=1,
                                name=f"maxacc_{r}_{g}_{qb}")
                maxred = small.tile([126, 1], F16, tag="maxred", bufs=3,
                                    name=f"maxred_{r}_{g}_{qb}")
                lhs = stg[(r, g, qb)]
                for ci, (c0, cw) in enumerate(_chunks(S, 2048)):
                    ps = psum.tile([126, 2048], F32, tag="ps",
                                   name=f"sc_ps_{r}_{g}_{qb}_{c0}")
                    for js, nw in _chunks(cw, 512):
                        nc.tensor.matmul(
                            ps[0:mp, js:js + nw],
                            lhs[0:kg, 0:mp],
                            kp[g][0:kg, c0 + js:c0 + js + nw],
                            start=True, stop=True,
                        )
                    nc.scalar.activation(
                        eb[0:mp, c0:c0 + cw], ps[0:mp, 0:cw],
                        mybir.ActivationFunctionType.Exp,
                        accum_out=parts[0:mp, ci:ci + 1],
                    )
                    # running max via tensor_tensor (2x-packed fp16 on
                    # DVE; tensor_reduce only has a 1x uop)
                    if ci == 1:
                        nc.vector.tensor_tensor(
                            acc[0:mp, 0:2048], eb[0:mp, 0:2048],
                            eb[0:mp, 2048:4096], op=mybir.AluOpType.max,
                        )
                    elif ci >= 2:
                        nc.vector.tensor_tensor(
                            acc[0:mp, 0:cw], acc[0:mp, 0:cw],
                            eb[0:mp, c0:c0 + cw], op=mybir.AluOpType.max,
                        )
                nc.vector.tensor_reduce(
                    sums[qb][0:mp, g:g + 1], parts[0:mp, 0:5],
                    axis=mybir.AxisListType.X, op=mybir.AluOpType.add,
                )
                nc.vector.tensor_reduce(
                    maxred[0:mp, 0:1], acc[0:mp, 0:2048],
                    axis=mybir.AxisListType.X, op=mybir.AluOpType.max,
                )
                qs = QBLOCKS[qb][1]
                for rr in range(hg):
                    nc.sync.dma_start(
                        out=gatherB[qb][0:qs, h0 + rr:h0 + rr + 1],
                        in_=maxred[rr * qs:rr * qs + qs, 0:1],
                    )

        # per-query totals -> rec; per-query max -> quant multiplier;
        # host dequant scale = qmax * rec / QLEV. Cross-partition moves are
        # tiny SBUF->SBUF DMAs (no PE/PSUM involvement -> no contention
        # with the next round's score matmuls).
        qmulP = {}
        for qb, (q0, qs) in enumerate(QBLOCKS):
            qa = r * ROUND_Q + q0
            sumg = small.tile([42, 8], F32, tag=f"sumg{qb}", bufs=1,
                              name=f"sumg_{r}_{qb}")
            for g, (hg, h0) in enumerate(HGROUPS):
                for rr in range(hg):
                    nc.sync.dma_start(
                        out=sumg[0:qs, h0 + rr:h0 + rr + 1],
                        in_=sums[qb][rr * qs:rr * qs + qs, g:g + 1],
                    )
            tot = small.tile([qs, 1], F32, tag="tot", name=f"tot_{r}_{qb}")
            nc.vector.tensor_reduce(tot[0:qs, 0:1], sumg[0:qs, 0:8],
                                    axis=mybir.AxisListType.X,
                                    op=mybir.AluOpType.add)
            rec = small.tile([qs, 1], F32, tag="rec", name=f"rec_{r}_{qb}")
            nc.vector.reciprocal(rec[0:qs, 0:1], tot[0:qs, 0:1])
            qmax16 = small.tile([qs, 1], F16, tag="qmax16",
                                name=f"qmax16_{r}_{qb}")
            nc.vector.tensor_reduce(qmax16[0:qs, 0:1],
                                    gatherB[qb][0:qs, 0:8],
                                    axis=mybir.AxisListType.X,
                                    op=mybir.AluOpType.max)
            qmax = small.tile([qs, 1], F32, tag="qmax", name=f"qmax_{r}_{qb}")
            nc.vector.tensor_copy(qmax[0:qs, 0:1], qmax16[0:qs, 0:1])
            invq = small.tile([qs, 1], F32, tag="invq", name=f"invq_{r}_{qb}")
            nc.vector.reciprocal(invq[0:qs, 0:1], qmax[0:qs, 0:1])
            qmul = small.tile([qs, 1], F32, tag="qmul", name=f"qmul_{r}_{qb}")
            nc.vector.tensor_scalar_mul(qmul[0:qs, 0:1], invq[0:qs, 0:1], QLEV)
            osc = small.tile([qs, 1], F32, tag="osc", bufs=2,
                             name=f"osc_{r}_{qb}")
            nc.vector.tensor_mul(osc[0:qs, 0:1], qmax[0:qs, 0:1],
                                 rec[0:qs, 0:1])
            nc.vector.tensor_scalar_mul(osc[0:qs, 0:1], osc[0:qs, 0:1],
                                        1.0 / QLEV)
            nc.sync.dma_start(out=d["oscale"][qa:qa + qs, 0:1],
                              in_=osc[0:qs, 0:1])
            qp_ = small.tile([126, 1], F32, tag=f"qmulP{qb}", bufs=2,
                             name=f"qmulP_{r}_{qb}")
            for rr in range(3):
                nc.sync.dma_start(
                    out=qp_[rr * qs:rr * qs + qs, 0:1],
                    in_=qmul[0:qs, 0:1],
                )
            qmulP[qb] = qp_

        # quantize (u8 = floor(exp * qmul + 0.5), per-query qmul) and write
        for qb, (q0, qs) in enumerate(QBLOCKS):
            qa = r * ROUND_Q + q0
            for g, (hg, h0) in enumerate(HGROUPS):
                mp = hg * qs
                eb = expb[(g, qb)]
                # quantize u8 = floor(exp * qmul + 0.5) on DVE (runs
                # fp16 2x-packed there, twice ACT's rate)
                for c0, cw in _chunks(S, 5000):
                    u8t = work.tile([126, 5000], U8, tag="u8d", bufs=2,
                                    name=f"u8d_{r}_{g}_{qb}_{c0}")
                    nc.vector.tensor_scalar(
                        u8t[0:mp, 0:cw], eb[0:mp, c0:c0 + cw],
                        qmulP[qb][0:mp, 0:1], 0.5,
                        op0=mybir.AluOpType.mult,
                        op1=mybir.AluOpType.add,
                    )
                    # one DMA per head-row-block: more, smaller dma_start
                    # calls spread the 12MB output across DMA engines
                    for rr in range(hg):
                        nc.sync.dma_start(
                            out=out_r[h0 + rr, qa:qa + qs, c0:c0 + cw],
                            in_=u8t[rr * qs:rr * qs + qs, 0:cw],
                        )


_CACHED = {}


def _build(use_qbias, use_kbias):
    key = (use_qbias, use_kbias)
    if key in _CACHED:
        return _CACHED[key]
    nc = bacc.Bacc("TRN2", target_bir_lowering=False, debug=False)
    d = {}
    d["qT"] = nc.dram_tensor("qT", [256, QS], F16, kind="ExternalInput")
    d["k"] = nc.dram_tensor("k", [256, S], F16, kind="ExternalInput")
    d["maskb"] = nc.dram_tensor("maskb", [1, S], F16, kind="ExternalInput")
    d["q_wT"] = nc.dram_tensor("q_wT", [256, 256], F16, kind="ExternalInput")
    d["k_wT"] = nc.dram_tensor("k_wT", [256, 256], F16, kind="ExternalInput")
    if use_qbias:
        for g, (hg, h0) in enumerate(HGROUPS):
            d[f"qbias{g}"] = nc.dram_tensor(f"qbias{g}", [32 * hg, 1], F32,
                                            kind="ExternalInput")
    if use_kbias:
        for g, (hg, h0) in enumerate(HGROUPS):
            d[f"kbias{g}"] = nc.dram_tensor(f"kbias{g}", [32 * hg, 1], F32,
                                            kind="ExternalInput")
    d["out"] = nc.dram_tensor("out", [QS, NH * S], U8, kind="ExternalOutput")
    d["oscale"] = nc.dram_tensor("oscale", [QS, 1], F32,
                                 kind="ExternalOutput")
    from contextlib import ExitStack
    with tile.TileContext(nc) as tc:
        with ExitStack() as ctx:
            _emit(nc, tc, ctx, d, use_qbias, use_kbias)
    nc.compile()
    _CACHED[key] = nc
    return nc


_POOL = None


def _pool():
    global _POOL
    if _POOL is None:
        _POOL = ThreadPoolExecutor(max_workers=NCORES)
    return _POOL


def make_in_maps(q, k, mask, q_w, q_b, k_w, k_b):
    use_qbias = bool(np.any(q_b != 0))
    use_kbias = bool(np.any(k_b != 0))
    shared = {
        "q_wT": np.ascontiguousarray(q_w.T, dtype=np.float16),
        "k_wT": np.ascontiguousarray(k_w.T, dtype=np.float16),
    }
    if use_qbias:
        qb_scaled = (q_b.astype(np.float32) * NORM_FACT).reshape(256, 1)
        for g, (hg, h0) in enumerate(HGROUPS):
            shared[f"qbias{g}"] = np.ascontiguousarray(
                qb_scaled[32 * h0:32 * h0 + 32 * hg])
    if use_kbias:
        kb_col = k_b.astype(np.float32).reshape(256, 1)
        for g, (hg, h0) in enumerate(HGROUPS):
            shared[f"kbias{g}"] = np.ascontiguousarray(
                kb_col[32 * h0:32 * h0 + 32 * hg])

    def percore(c):
        b, qh = divmod(c, 2)
        m = dict(shared)
        m["qT"] = np.ascontiguousarray(
            q[b, qh * QS:(qh + 1) * QS, :].T, dtype=np.float16)
        m["k"] = k[b].reshape(256, S).astype(np.float16)
        m["maskb"] = np.where(mask[b].reshape(1, S), np.float16(MASK_NEG),
                              np.float16(0.0))
        return m

    in_maps = list(_pool().map(percore, range(NCORES)))
    return in_maps, use_qbias, use_kbias


def assemble(results):
    """8 per-core {out: [QS, NH*S] u8, oscale: [QS,1] f32} -> [B,Q,NH,H,W]."""
    full = np.empty((B, Q, NH, H, W), np.float32)

    def percore(c):
        b, qh = divmod(c, 2)
        u8 = results[c]["out"].reshape(QS, NH, H, W)
        sc = np.ascontiguousarray(
            results[c]["oscale"], dtype=np.float32).reshape(QS, 1, 1, 1)
        np.multiply(u8, sc, out=full[b, qh * QS:(qh + 1) * QS])

    list(_pool().map(percore, range(NCORES)))
    return full


def kernel(q, k, mask, q_w, q_b, k_w, k_b, _trace=False):
    in_maps, use_qbias, use_kbias = make_in_maps(q, k, mask, q_w, q_b, k_w, k_b)
    nc = _build(use_qbias, use_kbias)
    res = run_bass_kernel_spmd(nc, in_maps, core_ids=list(range(NCORES)),
                               trace=_trace)
    out = assemble(res.results)
    if _trace:
        return out, res
    return out
